# revision 1
# baseline (speedup 1.0000x reference)
"""H2GCN forward, distributed across 8 NeuronCores.

Device (8-core row-sharded, via XLA on the NeuronCores): the dense stages —
embedding matmul + relu, and the final 448->16 classify matmul with
log_softmax. Host (CPU): the two sparse propagation hops (segment-sum
message passing), which the current neuronx-cc cannot compile (internal
compiler error on large gather ops).

Sharding: rows (nodes) are sharded across the 8 cores for both device
stages; the small weight matrices are replicated.
"""

import numpy as np
import jax
import jax.numpy as jnp
from jax.sharding import Mesh, PartitionSpec as P
from jax.experimental.shard_map import shard_map

N = 100000
FEAT = 512
HID = 64
CLS = 16
NCORES = 8

_mesh = None
_stage_a = None   # x_shard, w_embed -> relu(x @ w_embed)
_stage_b = None   # rfinal_shard, w_classify -> log_softmax(rfinal @ w_classify)
_cpu_hops = None


def _get_mesh():
    global _mesh
    if _mesh is None:
        devs = [d for d in jax.devices() if d.platform != "cpu"][:NCORES]
        if len(devs) < NCORES:   # no accelerators visible: degrade to CPU
            devs = jax.devices("cpu") * NCORES
        _mesh = Mesh(np.asarray(devs[:NCORES]), ("core",))
    return _mesh


def _build():
    global _stage_a, _stage_b, _cpu_hops
    if _stage_a is not None:
        return
    mesh = _get_mesh()

    def a_body(x, w_embed):
        return jax.nn.relu(x @ w_embed)

    def b_body(rfinal, w_classify):
        return jax.nn.log_softmax(rfinal @ w_classify, axis=1)

    _stage_a = jax.jit(shard_map(a_body, mesh=mesh,
                                 in_specs=(P("core"), P()),
                                 out_specs=P("core"), check_rep=False))
    _stage_b = jax.jit(shard_map(b_body, mesh=mesh,
                                 in_specs=(P("core"), P()),
                                 out_specs=P("core"), check_rep=False))

    cpu = jax.devices("cpu")[0]

    def spmm(idx, val, h):
        return jax.ops.segment_sum(val[:, None] * jnp.take(h, idx[1], axis=0),
                                   idx[0], num_segments=N)

    def hops(h, a1_idx, a1_val, a2_idx, a2_val):
        act = jax.nn.relu
        s1 = act(jnp.concatenate(
            [spmm(a1_idx, a1_val, h), spmm(a2_idx, a2_val, h)], axis=1))
        s2 = act(jnp.concatenate(
            [spmm(a1_idx, a1_val, s1), spmm(a2_idx, a2_val, s1)], axis=1))
        return s1, s2

    _cpu_hops = jax.jit(hops, device=cpu)


def kernel(x, a1_idx, a1_val, a2_idx, a2_val, w_embed, w_classify):
    _build()
    x = np.asarray(x, np.float32)
    a1_idx = np.asarray(a1_idx, np.int32)
    a2_idx = np.asarray(a2_idx, np.int32)
    a1_val = np.asarray(a1_val, np.float32)
    a2_val = np.asarray(a2_val, np.float32)
    w_embed = np.asarray(w_embed, np.float32)
    w_classify = np.asarray(w_classify, np.float32)

    h = np.asarray(_stage_a(x, w_embed).block_until_ready())       # [N, 64]
    s1, s2 = _cpu_hops(h, a1_idx, a1_val, a2_idx, a2_val)
    rfinal = np.concatenate([h, np.asarray(s1), np.asarray(s2)], axis=1)
    out = _stage_b(rfinal, w_classify)
    return np.asarray(out.block_until_ready())



# revision 16
# speedup vs baseline: 4.7499x; 4.7499x over previous
"""H2GCN forward on 8 Trainium2 NeuronCores via a single Bass kernel.

Sharding: destination nodes row-sharded across the 8 cores (12500 rows each).
Each core computes the embedding for its rows, AllGathers h so every core
holds the full node-feature table, then runs both propagation hops locally:
edges are grouped by (dest 128-row chunk, source 32k block), source rows are
fetched with dma_gather, scaled by edge values, and segment-summed into PSUM
with one-hot matmuls (the one-hot "P" matrices are built on the vector
engine with an is_equal compare against an iota row).  s1 is AllGathered
between the hops (the halo exchange).  The final 448->16 classify matmul and
log_softmax run on the local rows.  Everything below the embedding input is
bf16 with f32 PSUM accumulation.

The whole forward is ONE device dispatch per call.
"""

import sys
import numpy as np

sys.path.insert(0, "/opt/trn_rl_repo")

import ml_dtypes

bf16 = ml_dtypes.bfloat16

N = 100000
NC = 8
RPC = N // NC          # rows per core
CH = 128               # dest-chunk rows
NCH = (RPC + CH - 1) // CH   # 98 chunks per core (last has 84 rows)
FEAT = 512
HID = 64
CLS = 16
BLK = 32768            # source block size (int16 gather index limit)
NBLK = 4
E1 = 3_200_000
E2 = 6_400_000

_cache = {}
DEBUG_DUMPS = False


# ----------------------------------------------------------------- host prep

def _pack_adjacency(idx, val):
    """Split edges by (core, chunk, block); return per-core padded token
    streams (col_local, val, rowrel) plus the static per-block group counts.

    Token stream order per (core, chunk): block segments b0..b3, each padded
    to gb[b]*128 tokens (pad: col=0, val=0, rowrel=0)."""
    rows, cols = np.asarray(idx[0], np.int64), np.asarray(idx[1], np.int64)
    core = rows // RPC
    rloc = rows % RPC
    chunk = rloc // CH
    rowrel = rloc % CH
    b = cols >> 15
    col_local = (cols & (BLK - 1)).astype(np.int16)

    seg = ((core * NCH + chunk) * NBLK + b)
    order = np.argsort(seg, kind="stable")
    seg_s = seg[order]
    nseg = NC * NCH * NBLK
    cnt = np.bincount(seg_s, minlength=nseg)
    gb = (cnt.reshape(NC, NCH, NBLK).max(axis=(0, 1)) + 127) // 128  # [4]
    cap = (gb * 128).astype(np.int64)                                # [4]

    seg_start = np.zeros(nseg + 1, np.int64)
    np.cumsum(cnt, out=seg_start[1:])
    pad_start = np.tile(np.concatenate([[0], np.cumsum(cap)[:-1]]), NC * NCH)
    pad_start += np.repeat(np.arange(NC * NCH) * cap.sum(), NBLK)
    pos = np.arange(len(seg_s)) - seg_start[seg_s] + pad_start[seg_s]

    tot = NC * NCH * cap.sum()
    c_arr = np.zeros(tot, np.int16)
    v_arr = np.zeros(tot, bf16)
    r_arr = np.zeros(tot, bf16)
    c_arr[pos] = col_local[order]
    v_arr[pos] = np.asarray(val, np.float32)[order].astype(bf16)
    r_arr[pos] = rowrel[order].astype(np.float32).astype(bf16)
    per_core = tot // NC
    sh = (NC, NCH, per_core // NCH)
    return (c_arr.reshape(sh), v_arr.reshape(sh), r_arr.reshape(sh),
            tuple(int(g) for g in gb))


def _preprocess(x, a1_idx, a1_val, a2_idx, a2_val, w_embed, w_classify):
    c1, v1, r1, gb1 = _pack_adjacency(a1_idx, a1_val)
    c2, v2, r2, gb2 = _pack_adjacency(a2_idx, a2_val)
    G1, G2 = sum(gb1), sum(gb2)
    G = G1 + G2

    xT = np.asarray(x, np.float32).T.astype(bf16)          # [512, 100000]
    we = np.asarray(w_embed, np.float32).astype(bf16)      # [512, 64]
    wc = np.zeros((128, 4, CLS), bf16)
    wcf = np.asarray(w_classify, np.float32)
    wc[0:64, 0] = wcf[0:64].astype(bf16)
    wc[:, 1] = wcf[64:192].astype(bf16)
    wc[:, 2] = wcf[192:320].astype(bf16)
    wc[:, 3] = wcf[320:448].astype(bf16)

    in_maps = []
    for k in range(NC):
        # idx: per chunk [128, G*8] int16 (16-partition wrap, replicated x8)
        idx_c = np.concatenate([c1[k], c2[k]], axis=1)      # [NCH, G*128]
        idx_c = idx_c.reshape(NCH, G * 8, 16).transpose(0, 2, 1)  # [NCH,16,G*8]
        idx_c = np.tile(idx_c, (1, 8, 1))                   # [NCH,128,G*8]
        # meta: [NCH, 128, 2G] = [val | rowrel], token g*128+p at [p, g]
        v = np.concatenate([v1[k], v2[k]], axis=1).reshape(NCH, G, 128)
        r = np.concatenate([r1[k], r2[k]], axis=1).reshape(NCH, G, 128)
        meta = np.concatenate([v.transpose(0, 2, 1), r.transpose(0, 2, 1)],
                              axis=2)                       # [NCH,128,2G]
        in_maps.append({
            "xT": np.ascontiguousarray(xT[:, k * RPC:(k + 1) * RPC]),
            "we": we,
            "wc": wc,
            "edge_idx": np.ascontiguousarray(idx_c),
            "edge_meta": np.ascontiguousarray(meta),
        })
    return in_maps, (gb1, gb2)


# ------------------------------------------------------------- bass program

def _build_program(gb1, gb2):
    import concourse.bass as bass
    import concourse.mybir as mybir
    from concourse import bacc, tile

    G1, G2 = sum(gb1), sum(gb2)
    G = G1 + G2
    dt = mybir.dt

    nc = bacc.Bacc(None, target_bir_lowering=False)
    xT_t = nc.dram_tensor("xT", [FEAT, RPC], dt.bfloat16, kind="ExternalInput")
    we_t = nc.dram_tensor("we", [FEAT, HID], dt.bfloat16, kind="ExternalInput")
    wc_t = nc.dram_tensor("wc", [128, 4, CLS], dt.bfloat16, kind="ExternalInput")
    idx_t = nc.dram_tensor("edge_idx", [NCH, 128, G * 8], dt.int16,
                           kind="ExternalInput")
    meta_t = nc.dram_tensor("edge_meta", [NCH, 128, 2 * G], dt.bfloat16,
                            kind="ExternalInput")
    out_t = nc.dram_tensor("out", [RPC, CLS], dt.float32, kind="ExternalOutput")
    dbg = {}
    if DEBUG_DUMPS:
        dbg["dbg_h"] = nc.dram_tensor("dbg_h", [RPC, 128], dt.bfloat16,
                                      kind="ExternalOutput")
        dbg["dbg_s1"] = nc.dram_tensor("dbg_s1", [RPC, 128], dt.bfloat16,
                                       kind="ExternalOutput")
        dbg["dbg_s2"] = nc.dram_tensor("dbg_s2", [RPC, 256], dt.bfloat16,
                                       kind="ExternalOutput")

    with tile.TileContext(nc) as tc:
        with tc.tile_pool(name="dram", bufs=1, space="DRAM") as dram:
            h_loc = dram.tile([RPC, 128], dt.bfloat16)
            h_full = dram.tile([N, 128], dt.bfloat16, addr_space="Shared")
            s1_loc = dram.tile([RPC, 128], dt.bfloat16)
            s1_full = dram.tile([N, 128], dt.bfloat16, addr_space="Shared")
            s2_loc = dram.tile([RPC, 256], dt.bfloat16)

            _embed(nc, tc, xT_t, we_t, h_loc)
            nc.gpsimd.collective_compute(
                "AllGather", mybir.AluOpType.bypass,
                replica_groups=[list(range(NC))],
                ins=[h_loc[:]], outs=[h_full[:]])
            _hop(nc, tc, "h1", gb1, gb2, idx_t, meta_t, h_full, s1_loc,
                 src_w=64, dst_w=64)
            nc.gpsimd.collective_compute(
                "AllGather", mybir.AluOpType.bypass,
                replica_groups=[list(range(NC))],
                ins=[s1_loc[:]], outs=[s1_full[:]])
            _hop(nc, tc, "h2", gb1, gb2, idx_t, meta_t, s1_full, s2_loc,
                 src_w=128, dst_w=128)
            _classify(nc, tc, wc_t, h_loc, s1_loc, s2_loc, out_t)
            if DEBUG_DUMPS:
                nc.sync.dma_start(out=dbg["dbg_h"][:], in_=h_loc[:])
                nc.sync.dma_start(out=dbg["dbg_s1"][:], in_=s1_loc[:])
                nc.sync.dma_start(out=dbg["dbg_s2"][:], in_=s2_loc[:])

    nc.compile()
    return nc


def _embed(nc, tc, xT_t, we_t, h_loc):
    import concourse.mybir as mybir
    dt = mybir.dt
    with (
        tc.tile_pool(name="e_pool", bufs=3) as pool,
        tc.tile_pool(name="e_wp", bufs=1) as wp,
        tc.tile_pool(name="e_psum", bufs=2, space="PSUM") as psum,
    ):
        we_sb = wp.tile([128, 4, HID], dt.bfloat16)
        nc.sync.dma_start(out=we_sb[:],
                          in_=we_t.rearrange("(a p) h -> p a h", p=128))
        for t in range(NCH):
            r0 = t * CH
            R = min(CH, RPC - r0)
            xt_sb = pool.tile([128, 4, CH], dt.bfloat16, tag="xt")
            nc.sync.dma_start(
                out=xt_sb[:, :, 0:R],
                in_=xT_t.rearrange("(a p) r -> p a r", p=128)[:, :, r0:r0 + R])
            h_ps = psum.tile([CH, HID], dt.float32, tag="hps")
            for a in range(4):
                nc.tensor.matmul(h_ps[0:R], xt_sb[:, a, 0:R], we_sb[:, a, :],
                                 start=(a == 0), stop=(a == 3))
            h_sb = pool.tile([CH, 128], dt.bfloat16, tag="hsb")
            nc.scalar.activation(h_sb[0:R, 0:HID], h_ps[0:R],
                                 mybir.ActivationFunctionType.Relu)
            nc.sync.dma_start(out=h_loc[r0:r0 + R, :], in_=h_sb[0:R])


def _hop(nc, tc, name, gb1, gb2, idx_t, meta_t, src_full, dst_loc, src_w, dst_w):
    """One propagation hop: for each dest chunk, gather sources, scale,
    one-hot matmul-reduce (A1 -> psum[:, :dst_w], A2 -> psum[:, dst_w:]),
    relu, store."""
    import concourse.mybir as mybir
    dt = mybir.dt
    G1, G2 = sum(gb1), sum(gb2)
    G = G1 + G2
    with (
        tc.tile_pool(name=f"{name}_pool", bufs=2) as pool,
        tc.tile_pool(name=f"{name}_cpool", bufs=1) as cpool,
        tc.tile_pool(name=f"{name}_psum", bufs=2, space="PSUM") as psum,
    ):
        iota_sb = cpool.tile([128, 128], dt.bfloat16)
        nc.gpsimd.iota(iota_sb[:], pattern=[[1, 128]], base=0,
                       channel_multiplier=0,
                       allow_small_or_imprecise_dtypes=True)
        for c in range(NCH):
            r0 = c * CH
            R = min(CH, RPC - r0)
            idx_sb = pool.tile([128, G * 8], dt.int16, tag="idx")
            meta_sb = pool.tile([128, 2 * G], dt.bfloat16, tag="meta")
            gath = pool.tile([128, G, 128], dt.bfloat16, tag="gath")
            P = pool.tile([128, G, 128], dt.bfloat16, tag="P")
            nc.sync.dma_start(out=idx_sb[:], in_=idx_t[c])
            nc.sync.dma_start(out=meta_sb[:], in_=meta_t[c])
            off = 0
            for b, gb in enumerate(gb1 + gb2):
                while gb > 0:
                    g = min(gb, 8)   # 1024-token cap per dma_gather
                    nc.gpsimd.dma_gather(
                        out_ap=gath[:, off:off + g, :],
                        in_ap=src_full[b % NBLK * BLK:, :],
                        idxs_ap=idx_sb[:, off * 8:(off + g) * 8],
                        num_idxs=g * 128, num_idxs_reg=g * 128,
                        elem_size=128)
                    off += g
                    gb -= g
            nc.vector.tensor_tensor(
                out=P[:],
                in0=iota_sb[:, None, :].broadcast_to([128, G, 128]),
                in1=meta_sb[:, G:2 * G, None].broadcast_to([128, G, 128]),
                op=mybir.AluOpType.is_equal)
            nc.vector.tensor_tensor(
                out=gath[:],
                in0=gath[:],
                in1=meta_sb[:, 0:G, None].broadcast_to([128, G, 128]),
                op=mybir.AluOpType.mult)
            acc = psum.tile([CH, 2 * dst_w], dt.float32, tag="acc")
            for g in range(G):
                a1 = g < G1
                nc.tensor.matmul(
                    acc[:, 0:dst_w] if a1 else acc[:, dst_w:2 * dst_w],
                    P[:, g, :], gath[:, g, 0:src_w],
                    start=(g == 0 or g == G1),
                    stop=(g == G1 - 1 or g == G - 1))
            o_sb = pool.tile([CH, 2 * dst_w], dt.bfloat16, tag="osb")
            nc.scalar.activation(o_sb[0:R], acc[0:R],
                                 mybir.ActivationFunctionType.Relu)
            nc.sync.dma_start(out=dst_loc[r0:r0 + R, :], in_=o_sb[0:R])


def _classify(nc, tc, wc_t, h_loc, s1_loc, s2_loc, out_t):
    import concourse.mybir as mybir
    dt = mybir.dt
    with (
        tc.tile_pool(name="c_pool", bufs=3) as pool,
        tc.tile_pool(name="c_wp", bufs=1) as wp,
        tc.tile_pool(name="c_psum", bufs=2, space="PSUM") as psum,
    ):
        wc_sb = wp.tile([128, 4, CLS], dt.bfloat16)
        nc.sync.dma_start(out=wc_sb[:], in_=wc_t[:])
        ident = wp.tile([128, 128], dt.bfloat16)
        icol = wp.tile([128, 1], dt.float32)
        irow = wp.tile([128, 128], dt.bfloat16)
        nc.gpsimd.iota(irow[:], pattern=[[1, 128]], base=0,
                       channel_multiplier=0, allow_small_or_imprecise_dtypes=True)
        nc.gpsimd.iota(icol[:], pattern=[[1, 1]], base=0,
                       channel_multiplier=1, allow_small_or_imprecise_dtypes=True)
        nc.vector.tensor_scalar(ident[:], irow[:], icol[:], None,
                                mybir.AluOpType.is_equal)
        for t in range(NCH):
            r0 = t * CH
            R = min(CH, RPC - r0)
            h_sb = pool.tile([CH, HID], dt.bfloat16, tag="chs")
            s1_sb = pool.tile([CH, 128], dt.bfloat16, tag="cs1")
            s2_sb = pool.tile([CH, 256], dt.bfloat16, tag="cs2")
            if R < CH:  # last partial tile: clear stale rows
                nc.vector.memset(h_sb[:], 0.0)
                nc.vector.memset(s1_sb[:], 0.0)
                nc.vector.memset(s2_sb[:], 0.0)
            nc.sync.dma_start(out=h_sb[0:R], in_=h_loc[r0:r0 + R, 0:HID])
            nc.sync.dma_start(out=s1_sb[0:R], in_=s1_loc[r0:r0 + R, :])
            nc.sync.dma_start(out=s2_sb[0:R], in_=s2_loc[r0:r0 + R, :])
            tp_ps = psum.tile([128, 4, CH], dt.bfloat16, tag="ctp")
            nc.tensor.transpose(tp_ps[0:HID, 0, :], h_sb[:], ident[:])
            nc.tensor.transpose(tp_ps[:, 1, :], s1_sb[:], ident[:])
            nc.tensor.transpose(tp_ps[:, 2, :], s2_sb[:, 0:128], ident[:])
            nc.tensor.transpose(tp_ps[:, 3, :], s2_sb[:, 128:256], ident[:])
            tp = pool.tile([128, 4, CH], dt.bfloat16, tag="ctps")
            nc.scalar.activation(tp[0:HID, 0, :], tp_ps[0:HID, 0, :],
                                 mybir.ActivationFunctionType.Copy)
            nc.scalar.activation(tp[:, 1:4, :], tp_ps[:, 1:4, :],
                                 mybir.ActivationFunctionType.Copy)
            lg = psum.tile([CH, CLS], dt.float32, tag="clg")
            for j in range(4):
                cdim = HID if j == 0 else 128
                nc.tensor.matmul(lg[:], tp[0:cdim, j, :], wc_sb[0:cdim, j, :],
                                 start=(j == 0), stop=(j == 3))
            mx = pool.tile([CH, 1], dt.float32, tag="cmx")
            tt = pool.tile([CH, CLS], dt.float32, tag="ctt")
            ee = pool.tile([CH, CLS], dt.float32, tag="cee")
            sm = pool.tile([CH, 1], dt.float32, tag="csm")
            ll = pool.tile([CH, 1], dt.float32, tag="cll")
            oo = pool.tile([CH, CLS], dt.float32, tag="coo")
            nc.vector.tensor_reduce(mx[:], lg[:], mybir.AxisListType.X,
                                    mybir.AluOpType.max)
            nc.vector.tensor_scalar(tt[:], lg[:], mx[:], None,
                                    mybir.AluOpType.subtract)
            nc.scalar.activation(ee[:], tt[:],
                                 mybir.ActivationFunctionType.Exp,
                                 accum_out=sm[:])
            nc.scalar.activation(ll[:], sm[:],
                                 mybir.ActivationFunctionType.Ln)
            nc.vector.tensor_scalar(oo[:], tt[:], ll[:], None,
                                    mybir.AluOpType.subtract)
            nc.sync.dma_start(out=out_t[r0:r0 + R, :], in_=oo[0:R])


# ---------------------------------------------------------------- execution

def _get_runner(key):
    """Build (once) the compiled SPMD callable for the given group counts.

    Returns dict with the jitted sharded fn and metadata. The callable takes
    (concat_inputs..., concat_zero_outputs...) and returns concat outputs."""
    if key in _cache:
        return _cache[key]
    import jax
    import concourse.bass2jax as bass2jax
    from concourse.bass2jax import _bass_exec_p, partition_id_tensor
    from jax.sharding import Mesh, PartitionSpec
    from jax.experimental.shard_map import shard_map

    gb1, gb2 = key
    nc = _build_program(gb1, gb2)
    bass2jax.install_neuronx_cc_hook()

    partition_name = (nc.partition_id_tensor.name
                      if nc.partition_id_tensor else None)
    in_names = ["xT", "we", "wc", "edge_idx", "edge_meta"]
    out_names = ["out"]
    out_avals = [jax.core.ShapedArray((RPC, CLS), np.float32)]
    if DEBUG_DUMPS:
        out_names += ["dbg_h", "dbg_s1", "dbg_s2"]
        out_avals += [jax.core.ShapedArray((RPC, 128), ml_dtypes.bfloat16),
                      jax.core.ShapedArray((RPC, 128), ml_dtypes.bfloat16),
                      jax.core.ShapedArray((RPC, 256), ml_dtypes.bfloat16)]
    all_names = in_names + out_names + ([partition_name] if partition_name else [])
    n_params = len(in_names)

    def _body(*args):
        operands = list(args)
        if partition_name is not None:
            operands.append(partition_id_tensor())
        outs = _bass_exec_p.bind(
            *operands,
            out_avals=tuple(out_avals),
            in_names=tuple(all_names),
            out_names=tuple(out_names),
            lowering_input_output_aliases=(),
            sim_require_finite=True, sim_require_nnan=True, nc=nc)
        return tuple(outs)

    def _body_k(k):
        def f(*args):
            ins, z = args[:n_params], args[n_params:]
            outs = _body(*ins, *z)
            for _ in range(k - 1):
                z2 = tuple(o * 0 for o in outs)
                outs = _body(*ins, *z2)
            return outs
        return f

    devices = jax.devices()[:NC]
    mesh = Mesh(np.asarray(devices), ("core",))
    n_outs = len(out_names)

    def _sharded(fn):
        return jax.jit(
            shard_map(fn, mesh=mesh,
                      in_specs=(PartitionSpec("core"),) * (n_params + n_outs),
                      out_specs=(PartitionSpec("core"),) * n_outs,
                      check_rep=False),
            donate_argnums=tuple(range(n_params, n_params + n_outs)),
            keep_unused=True)

    runner = {
        "mesh": mesh,
        "n_params": n_params,
        "in_names": in_names,
        "out_avals": out_avals,
        "fn": _sharded(_body),
        "fn_k": lambda k: _sharded(_body_k(k)),
    }
    _cache[key] = runner
    return runner


def _concat_inputs(in_maps, in_names):
    return [np.concatenate([in_maps[c][n] for c in range(NC)], axis=0)
            for n in in_names]


def kernel(x, a1_idx, a1_val, a2_idx, a2_val, w_embed, w_classify):
    in_maps, key = _preprocess(x, a1_idx, a1_val, a2_idx, a2_val,
                               w_embed, w_classify)
    runner = _get_runner(key)
    concat_in = _concat_inputs(in_maps, runner["in_names"])
    zeros = [np.zeros((NC * a.shape[0], *a.shape[1:]), a.dtype)
             for a in runner["out_avals"]]
    out = runner["fn"](*concat_in, *zeros)
    if DEBUG_DUMPS:
        return tuple(np.asarray(o) for o in out)
    return np.asarray(out[0])


# revision 30
# speedup vs baseline: 8.8635x; 1.8660x over previous
"""H2GCN forward on 8 Trainium2 NeuronCores via a single Bass kernel.

Sharding: destination nodes row-sharded across the 8 cores (12500 rows each).
Each core computes the embedding for its rows, AllGathers h so every core
holds the full node-feature table, then runs both propagation hops locally:
edges are grouped by (dest 128-row chunk, source 32k block), source rows are
fetched with dma_gather, scaled by edge values, and segment-summed into PSUM
with one-hot matmuls (the one-hot "P" matrices are built on the vector
engine with an is_equal compare against an iota row).  s1 is AllGathered
between the hops (the halo exchange).  The final 448->16 classify matmul and
log_softmax run on the local rows.  Everything below the embedding input is
bf16 with f32 PSUM accumulation.

The whole forward is ONE device dispatch per call.
"""

import sys
import numpy as np

sys.path.insert(0, "/opt/trn_rl_repo")

import ml_dtypes

bf16 = ml_dtypes.bfloat16

N = 100000
NC = 8
RPC = N // NC          # rows per core
CH = 128               # dest-chunk rows
NCH = (RPC + CH - 1) // CH   # 98 chunks per core (last has 84 rows)
FEAT = 512
HID = 64
CLS = 16
BLK = 32768            # source block size (int16 gather index limit)
NBLK = 4
E1 = 3_200_000
E2 = 6_400_000

_cache = {}
DEBUG_DUMPS = False
STAGES = "full"   # "embed" | "agh" | "hop1" | "ags1" | "hop2" | "full"
HOP_MODE = "full"  # "full" | "gather_only" | "no_gather"
NQUEUES = 2        # SWDGE queues to spread gathers over
SINGLE_PACKET = False
_STAGE_ORDER = ["embed", "agh", "hop1", "ags1", "hop2", "full"]


def _stage_on(s):
    return _STAGE_ORDER.index(STAGES) >= _STAGE_ORDER.index(s)


# ----------------------------------------------------------------- host prep

def _pack_adjacency(idx, val):
    """Split edges by (core, chunk, block); return per-core padded token
    streams (col_local, val, rowrel) plus the static per-block group counts.

    Token stream order per (core, chunk): block segments b0..b3, each padded
    to gb[b]*128 tokens (pad: col=0, val=0, rowrel=0)."""
    rows, cols = np.asarray(idx[0], np.int64), np.asarray(idx[1], np.int64)
    core = rows // RPC
    rloc = rows % RPC
    chunk = rloc // CH
    rowrel = rloc % CH
    b = cols >> 15
    col_local = (cols & (BLK - 1)).astype(np.int16)

    seg = ((core * NCH + chunk) * NBLK + b)
    order = np.argsort(seg, kind="stable")
    seg_s = seg[order]
    nseg = NC * NCH * NBLK
    cnt = np.bincount(seg_s, minlength=nseg)
    gb = (cnt.reshape(NC, NCH, NBLK).max(axis=(0, 1)) + 127) // 128  # [4]
    cap = (gb * 128).astype(np.int64)                                # [4]

    seg_start = np.zeros(nseg + 1, np.int64)
    np.cumsum(cnt, out=seg_start[1:])
    pad_start = np.tile(np.concatenate([[0], np.cumsum(cap)[:-1]]), NC * NCH)
    pad_start += np.repeat(np.arange(NC * NCH) * cap.sum(), NBLK)
    pos = np.arange(len(seg_s)) - seg_start[seg_s] + pad_start[seg_s]

    tot = NC * NCH * cap.sum()
    c_arr = np.zeros(tot, np.int16)
    v_arr = np.zeros(tot, bf16)
    r_arr = np.zeros(tot, bf16)
    c_arr[pos] = col_local[order]
    v_arr[pos] = np.asarray(val, np.float32)[order].astype(bf16)
    r_arr[pos] = rowrel[order].astype(np.float32).astype(bf16)
    per_core = tot // NC
    sh = (NC, NCH, per_core // NCH)
    return (c_arr.reshape(sh), v_arr.reshape(sh), r_arr.reshape(sh),
            tuple(int(g) for g in gb))


def _preprocess(x, a1_idx, a1_val, a2_idx, a2_val, w_embed, w_classify):
    c1, v1, r1, gb1 = _pack_adjacency(a1_idx, a1_val)
    c2, v2, r2, gb2 = _pack_adjacency(a2_idx, a2_val)
    G1, G2 = sum(gb1), sum(gb2)
    G = G1 + G2

    xT = np.asarray(x, np.float32).T.astype(bf16)          # [512, 100000]
    we = np.asarray(w_embed, np.float32).astype(bf16)      # [512, 64]
    wc = np.zeros((128, 4, CLS), bf16)
    wcf = np.asarray(w_classify, np.float32)
    wc[0:64, 0] = wcf[0:64].astype(bf16)
    wc[:, 1] = wcf[64:192].astype(bf16)
    wc[:, 2] = wcf[192:320].astype(bf16)
    wc[:, 3] = wcf[320:448].astype(bf16)

    in_maps = []
    for k in range(NC):
        # idx: per chunk [128, G*8] int16 (16-partition wrap, replicated x8)
        idx_c = np.concatenate([c1[k], c2[k]], axis=1)      # [NCH, G*128]
        idx_c = idx_c.reshape(NCH, G * 8, 16).transpose(0, 2, 1)  # [NCH,16,G*8]
        idx_c = np.tile(idx_c, (1, 8, 1))                   # [NCH,128,G*8]
        # meta: [NCH, 128, 2G] = [val | rowrel], token g*128+p at [p, g]
        v = np.concatenate([v1[k], v2[k]], axis=1).reshape(NCH, G, 128)
        r = np.concatenate([r1[k], r2[k]], axis=1).reshape(NCH, G, 128)
        meta = np.concatenate([v.transpose(0, 2, 1), r.transpose(0, 2, 1)],
                              axis=2)                       # [NCH,128,2G]
        in_maps.append({
            "xT": np.ascontiguousarray(xT[:, k * RPC:(k + 1) * RPC]),
            "we": we,
            "wc": wc,
            "edge_idx": np.ascontiguousarray(idx_c),
            "edge_meta": np.ascontiguousarray(meta),
        })
    return in_maps, (gb1, gb2)


# ------------------------------------------------------------- bass program

def _build_program(gb1, gb2):
    import concourse.bass as bass
    import concourse.mybir as mybir
    from concourse import bacc, tile

    G1, G2 = sum(gb1), sum(gb2)
    G = G1 + G2
    dt = mybir.dt

    nc = bacc.Bacc(None, target_bir_lowering=False,
                   num_swdge_queues=NQUEUES)
    xT_t = nc.dram_tensor("xT", [FEAT, RPC], dt.bfloat16, kind="ExternalInput")
    we_t = nc.dram_tensor("we", [FEAT, HID], dt.bfloat16, kind="ExternalInput")
    wc_t = nc.dram_tensor("wc", [128, 4, CLS], dt.bfloat16, kind="ExternalInput")
    idx_t = nc.dram_tensor("edge_idx", [NCH, 128, G * 8], dt.int16,
                           kind="ExternalInput")
    meta_t = nc.dram_tensor("edge_meta", [NCH, 128, 2 * G], dt.bfloat16,
                            kind="ExternalInput")
    out_t = nc.dram_tensor("out", [RPC, CLS], dt.float32, kind="ExternalOutput")
    dbg = {}
    if DEBUG_DUMPS:
        dbg["dbg_h"] = nc.dram_tensor("dbg_h", [RPC, 128], dt.bfloat16,
                                      kind="ExternalOutput")
        dbg["dbg_s1"] = nc.dram_tensor("dbg_s1", [RPC, 128], dt.bfloat16,
                                       kind="ExternalOutput")
        dbg["dbg_s2"] = nc.dram_tensor("dbg_s2", [RPC, 256], dt.bfloat16,
                                       kind="ExternalOutput")

    with tile.TileContext(nc) as tc:
        with tc.tile_pool(name="dram", bufs=1, space="DRAM") as dram:
            h_loc = dram.tile([RPC, 128], dt.bfloat16)
            h_full = dram.tile([N, 128], dt.bfloat16, addr_space="Shared")
            s1_loc = dram.tile([RPC, 128], dt.bfloat16)
            s1_full = dram.tile([N, 128], dt.bfloat16, addr_space="Shared")
            s2_loc = dram.tile([RPC, 256], dt.bfloat16)

            _embed(nc, tc, xT_t, we_t, h_loc)
            if _stage_on("agh"):
                nc.gpsimd.collective_compute(
                    "AllGather", mybir.AluOpType.bypass,
                    replica_groups=[list(range(NC))],
                    ins=[h_loc[:]], outs=[h_full[:]])
            if _stage_on("hop1"):
                _hop(nc, tc, "h1", gb1, gb2, idx_t, meta_t, h_full, s1_loc,
                     src_w=64, dst_w=64)
            if _stage_on("ags1"):
                nc.gpsimd.collective_compute(
                    "AllGather", mybir.AluOpType.bypass,
                    replica_groups=[list(range(NC))],
                    ins=[s1_loc[:]], outs=[s1_full[:]])
            if _stage_on("hop2"):
                _hop(nc, tc, "h2", gb1, gb2, idx_t, meta_t, s1_full, s2_loc,
                     src_w=128, dst_w=128)
            if _stage_on("full"):
                _classify(nc, tc, wc_t, h_loc, s1_loc, s2_loc, out_t)
            if DEBUG_DUMPS:
                nc.sync.dma_start(out=dbg["dbg_h"][:], in_=h_loc[:])
                nc.sync.dma_start(out=dbg["dbg_s1"][:], in_=s1_loc[:])
                nc.sync.dma_start(out=dbg["dbg_s2"][:], in_=s2_loc[:])

    nc.compile()
    return nc


def _embed(nc, tc, xT_t, we_t, h_loc):
    import concourse.mybir as mybir
    dt = mybir.dt
    with (
        tc.tile_pool(name="e_pool", bufs=3) as pool,
        tc.tile_pool(name="e_wp", bufs=1) as wp,
        tc.tile_pool(name="e_psum", bufs=2, space="PSUM") as psum,
    ):
        we_sb = wp.tile([128, 4, HID], dt.bfloat16)
        nc.sync.dma_start(out=we_sb[:],
                          in_=we_t.rearrange("(a p) h -> p a h", p=128))
        for t in range(NCH):
            r0 = t * CH
            R = min(CH, RPC - r0)
            xt_sb = pool.tile([128, 4, CH], dt.bfloat16, tag="xt")
            nc.sync.dma_start(
                out=xt_sb[:, :, 0:R],
                in_=xT_t.rearrange("(a p) r -> p a r", p=128)[:, :, r0:r0 + R])
            h_ps = psum.tile([CH, HID], dt.float32, tag="hps")
            for a in range(4):
                nc.tensor.matmul(h_ps[0:R], xt_sb[:, a, 0:R], we_sb[:, a, :],
                                 start=(a == 0), stop=(a == 3))
            h_sb = pool.tile([CH, 128], dt.bfloat16, tag="hsb")
            nc.scalar.activation(h_sb[0:R, 0:HID], h_ps[0:R],
                                 mybir.ActivationFunctionType.Relu)
            nc.sync.dma_start(out=h_loc[r0:r0 + R, :], in_=h_sb[0:R])


def _hop(nc, tc, name, gb1, gb2, idx_t, meta_t, src_full, dst_loc, src_w, dst_w):
    """One propagation hop: for each dest chunk, gather sources, scale,
    one-hot matmul-reduce (A1 -> psum[:, :dst_w], A2 -> psum[:, dst_w:]),
    relu, store."""
    import concourse.mybir as mybir
    dt = mybir.dt
    G1, G2 = sum(gb1), sum(gb2)
    G = G1 + G2
    with (
        tc.tile_pool(name=f"{name}_pool", bufs=2) as pool,
        tc.tile_pool(name=f"{name}_cpool", bufs=1) as cpool,
        tc.tile_pool(name=f"{name}_psum", bufs=2, space="PSUM") as psum,
    ):
        iota_sb = cpool.tile([128, 128], dt.bfloat16)
        nc.gpsimd.iota(iota_sb[:], pattern=[[1, 128]], base=0,
                       channel_multiplier=0,
                       allow_small_or_imprecise_dtypes=True)
        for c in range(NCH):
            r0 = c * CH
            R = min(CH, RPC - r0)
            idx_sb = pool.tile([128, G * 8], dt.int16, tag="idx")
            meta_sb = pool.tile([128, 2 * G], dt.bfloat16, tag="meta")
            gath = pool.tile([128, G, 128], dt.bfloat16, tag="gath")
            P = pool.tile([128, G, 128], dt.bfloat16, tag="P")
            nc.sync.dma_start(out=idx_sb[:], in_=idx_t[c])
            nc.sync.dma_start(out=meta_sb[:], in_=meta_t[c])
            off = 0
            qn = 0
            for b, gb in enumerate(gb1 + gb2):
                while gb > 0:
                    g = gb if not SINGLE_PACKET else min(gb, 8)
                    if HOP_MODE != "no_gather":
                        nc.gpsimd.dma_gather(
                            out_ap=gath[:, off:off + g, :],
                            in_ap=src_full[b % NBLK * BLK:, :],
                            idxs_ap=idx_sb[:, off * 8:(off + g) * 8],
                            num_idxs=g * 128, num_idxs_reg=g * 128,
                            elem_size=128, queue_num=qn % NQUEUES,
                            single_packet=SINGLE_PACKET)
                        qn += 1
                    off += g
                    gb -= g
            if HOP_MODE == "no_gather":
                nc.vector.memset(gath[:, 0:1, :], 0.0)
            if HOP_MODE == "gather_only":
                continue
            nc.vector.tensor_tensor(
                out=P[:],
                in0=iota_sb[:, None, :].broadcast_to([128, G, 128]),
                in1=meta_sb[:, G:2 * G, None].broadcast_to([128, G, 128]),
                op=mybir.AluOpType.is_equal)
            nc.vector.tensor_tensor(
                out=gath[:],
                in0=gath[:],
                in1=meta_sb[:, 0:G, None].broadcast_to([128, G, 128]),
                op=mybir.AluOpType.mult)
            acc = psum.tile([CH, 2 * dst_w], dt.float32, tag="acc")
            for g in range(G):
                a1 = g < G1
                nc.tensor.matmul(
                    acc[:, 0:dst_w] if a1 else acc[:, dst_w:2 * dst_w],
                    P[:, g, :], gath[:, g, 0:src_w],
                    start=(g == 0 or g == G1),
                    stop=(g == G1 - 1 or g == G - 1))
            o_sb = pool.tile([CH, 2 * dst_w], dt.bfloat16, tag="osb")
            nc.scalar.activation(o_sb[0:R], acc[0:R],
                                 mybir.ActivationFunctionType.Relu)
            nc.sync.dma_start(out=dst_loc[r0:r0 + R, :], in_=o_sb[0:R])


def _classify(nc, tc, wc_t, h_loc, s1_loc, s2_loc, out_t):
    import concourse.mybir as mybir
    dt = mybir.dt
    with (
        tc.tile_pool(name="c_pool", bufs=3) as pool,
        tc.tile_pool(name="c_wp", bufs=1) as wp,
        tc.tile_pool(name="c_psum", bufs=2, space="PSUM") as psum,
    ):
        wc_sb = wp.tile([128, 4, CLS], dt.bfloat16)
        nc.sync.dma_start(out=wc_sb[:], in_=wc_t[:])
        ident = wp.tile([128, 128], dt.bfloat16)
        icol = wp.tile([128, 1], dt.float32)
        irow = wp.tile([128, 128], dt.bfloat16)
        nc.gpsimd.iota(irow[:], pattern=[[1, 128]], base=0,
                       channel_multiplier=0, allow_small_or_imprecise_dtypes=True)
        nc.gpsimd.iota(icol[:], pattern=[[1, 1]], base=0,
                       channel_multiplier=1, allow_small_or_imprecise_dtypes=True)
        nc.vector.tensor_scalar(ident[:], irow[:], icol[:], None,
                                mybir.AluOpType.is_equal)
        for t in range(NCH):
            r0 = t * CH
            R = min(CH, RPC - r0)
            h_sb = pool.tile([CH, HID], dt.bfloat16, tag="chs")
            s1_sb = pool.tile([CH, 128], dt.bfloat16, tag="cs1")
            s2_sb = pool.tile([CH, 256], dt.bfloat16, tag="cs2")
            if R < CH:  # last partial tile: clear stale rows
                nc.vector.memset(h_sb[:], 0.0)
                nc.vector.memset(s1_sb[:], 0.0)
                nc.vector.memset(s2_sb[:], 0.0)
            nc.sync.dma_start(out=h_sb[0:R], in_=h_loc[r0:r0 + R, 0:HID])
            nc.sync.dma_start(out=s1_sb[0:R], in_=s1_loc[r0:r0 + R, :])
            nc.sync.dma_start(out=s2_sb[0:R], in_=s2_loc[r0:r0 + R, :])
            tp_ps = psum.tile([128, 4, CH], dt.bfloat16, tag="ctp")
            nc.tensor.transpose(tp_ps[0:HID, 0, :], h_sb[:], ident[:])
            nc.tensor.transpose(tp_ps[:, 1, :], s1_sb[:], ident[:])
            nc.tensor.transpose(tp_ps[:, 2, :], s2_sb[:, 0:128], ident[:])
            nc.tensor.transpose(tp_ps[:, 3, :], s2_sb[:, 128:256], ident[:])
            tp = pool.tile([128, 4, CH], dt.bfloat16, tag="ctps")
            nc.scalar.activation(tp[0:HID, 0, :], tp_ps[0:HID, 0, :],
                                 mybir.ActivationFunctionType.Copy)
            nc.scalar.activation(tp[:, 1:4, :], tp_ps[:, 1:4, :],
                                 mybir.ActivationFunctionType.Copy)
            lg = psum.tile([CH, CLS], dt.float32, tag="clg")
            for j in range(4):
                cdim = HID if j == 0 else 128
                nc.tensor.matmul(lg[:], tp[0:cdim, j, :], wc_sb[0:cdim, j, :],
                                 start=(j == 0), stop=(j == 3))
            mx = pool.tile([CH, 1], dt.float32, tag="cmx")
            tt = pool.tile([CH, CLS], dt.float32, tag="ctt")
            ee = pool.tile([CH, CLS], dt.float32, tag="cee")
            sm = pool.tile([CH, 1], dt.float32, tag="csm")
            ll = pool.tile([CH, 1], dt.float32, tag="cll")
            oo = pool.tile([CH, CLS], dt.float32, tag="coo")
            nc.vector.tensor_reduce(mx[:], lg[:], mybir.AxisListType.X,
                                    mybir.AluOpType.max)
            nc.vector.tensor_scalar(tt[:], lg[:], mx[:], None,
                                    mybir.AluOpType.subtract)
            nc.scalar.activation(ee[:], tt[:],
                                 mybir.ActivationFunctionType.Exp,
                                 accum_out=sm[:])
            nc.scalar.activation(ll[:], sm[:],
                                 mybir.ActivationFunctionType.Ln)
            nc.vector.tensor_scalar(oo[:], tt[:], ll[:], None,
                                    mybir.AluOpType.subtract)
            nc.sync.dma_start(out=out_t[r0:r0 + R, :], in_=oo[0:R])


# ---------------------------------------------------------------- execution

def _get_runner(key):
    key = (key, STAGES, DEBUG_DUMPS, HOP_MODE)
    return _get_runner_impl(key)


def _get_runner_impl(key):
    """Build (once) the compiled SPMD callable for the given group counts.

    Returns dict with the jitted sharded fn and metadata. The callable takes
    (concat_inputs..., concat_zero_outputs...) and returns concat outputs."""
    if key in _cache:
        return _cache[key]
    import jax
    import concourse.bass2jax as bass2jax
    from concourse.bass2jax import _bass_exec_p, partition_id_tensor
    from jax.sharding import Mesh, PartitionSpec
    from jax.experimental.shard_map import shard_map

    (gb1, gb2), _, _, _ = key
    nc = _build_program(gb1, gb2)
    bass2jax.install_neuronx_cc_hook()

    partition_name = (nc.partition_id_tensor.name
                      if nc.partition_id_tensor else None)
    in_names = ["xT", "we", "wc", "edge_idx", "edge_meta"]
    out_names = ["out"]
    out_avals = [jax.core.ShapedArray((RPC, CLS), np.float32)]
    if DEBUG_DUMPS:
        out_names += ["dbg_h", "dbg_s1", "dbg_s2"]
        out_avals += [jax.core.ShapedArray((RPC, 128), ml_dtypes.bfloat16),
                      jax.core.ShapedArray((RPC, 128), ml_dtypes.bfloat16),
                      jax.core.ShapedArray((RPC, 256), ml_dtypes.bfloat16)]
    all_names = in_names + out_names + ([partition_name] if partition_name else [])
    n_params = len(in_names)

    def _body(*args):
        operands = list(args)
        if partition_name is not None:
            operands.append(partition_id_tensor())
        outs = _bass_exec_p.bind(
            *operands,
            out_avals=tuple(out_avals),
            in_names=tuple(all_names),
            out_names=tuple(out_names),
            lowering_input_output_aliases=(),
            sim_require_finite=True, sim_require_nnan=True, nc=nc)
        return tuple(outs)

    def _body_k(k):
        def f(*args):
            ins, z = args[:n_params], args[n_params:]
            outs = _body(*ins, *z)
            for _ in range(k - 1):
                z2 = tuple(o * 0 for o in outs)
                outs = _body(*ins, *z2)
            return outs
        return f

    devices = jax.devices()[:NC]
    mesh = Mesh(np.asarray(devices), ("core",))
    n_outs = len(out_names)

    def _sharded(fn):
        return jax.jit(
            shard_map(fn, mesh=mesh,
                      in_specs=(PartitionSpec("core"),) * (n_params + n_outs),
                      out_specs=(PartitionSpec("core"),) * n_outs,
                      check_rep=False),
            donate_argnums=tuple(range(n_params, n_params + n_outs)),
            keep_unused=True)

    runner = {
        "mesh": mesh,
        "n_params": n_params,
        "in_names": in_names,
        "out_avals": out_avals,
        "fn": _sharded(_body),
        "fn_k": lambda k: _sharded(_body_k(k)),
    }
    _cache[key] = runner
    return runner


def _concat_inputs(in_maps, in_names):
    return [np.concatenate([in_maps[c][n] for c in range(NC)], axis=0)
            for n in in_names]


def kernel(x, a1_idx, a1_val, a2_idx, a2_val, w_embed, w_classify):
    in_maps, key = _preprocess(x, a1_idx, a1_val, a2_idx, a2_val,
                               w_embed, w_classify)
    runner = _get_runner(key)
    concat_in = _concat_inputs(in_maps, runner["in_names"])
    zeros = [np.zeros((NC * a.shape[0], *a.shape[1:]), a.dtype)
             for a in runner["out_avals"]]
    out = runner["fn"](*concat_in, *zeros)
    if DEBUG_DUMPS:
        return tuple(np.asarray(o) for o in out)
    return np.asarray(out[0])


# revision 32
# speedup vs baseline: 8.9490x; 1.0096x over previous
"""H2GCN forward on 8 Trainium2 NeuronCores via a single Bass kernel.

Sharding: destination nodes row-sharded across the 8 cores (12500 rows each).
Each core computes the embedding for its rows, AllGathers h so every core
holds the full node-feature table, then runs both propagation hops locally:
edges are grouped by (dest 128-row chunk, source 32k block), source rows are
fetched with dma_gather, scaled by edge values, and segment-summed into PSUM
with one-hot matmuls (the one-hot "P" matrices are built on the vector
engine with an is_equal compare against an iota row).  s1 is AllGathered
between the hops (the halo exchange).  The final 448->16 classify matmul and
log_softmax run on the local rows.  Everything below the embedding input is
bf16 with f32 PSUM accumulation.

The whole forward is ONE device dispatch per call.
"""

import sys
import numpy as np

sys.path.insert(0, "/opt/trn_rl_repo")

import ml_dtypes

bf16 = ml_dtypes.bfloat16

N = 100000
NC = 8
RPC = N // NC          # rows per core
CH = 128               # dest-chunk rows
NCH = (RPC + CH - 1) // CH   # 98 chunks per core (last has 84 rows)
FEAT = 512
HID = 64
CLS = 16
BLK = 32768            # source block size (int16 gather index limit)
NBLK = 4
E1 = 3_200_000
E2 = 6_400_000

_cache = {}
DEBUG_DUMPS = False
STAGES = "full"   # "embed" | "agh" | "hop1" | "ags1" | "hop2" | "full"
HOP_MODE = "full"  # "full" | "gather_only" | "no_gather"
NQUEUES = 2        # SWDGE queues to spread gathers over
HOPBUFS = 3        # gather/P pool depth (cross-chunk pipelining)
SINGLE_PACKET = False
_STAGE_ORDER = ["embed", "agh", "hop1", "ags1", "hop2", "full"]


def _stage_on(s):
    return _STAGE_ORDER.index(STAGES) >= _STAGE_ORDER.index(s)


# ----------------------------------------------------------------- host prep

def _pack_adjacency(idx, val):
    """Split edges by (core, chunk, block); return per-core padded token
    streams (col_local, val, rowrel) plus the static per-block group counts.

    Token stream order per (core, chunk): block segments b0..b3, each padded
    to gb[b]*128 tokens (pad: col=0, val=0, rowrel=0)."""
    rows, cols = np.asarray(idx[0], np.int64), np.asarray(idx[1], np.int64)
    core = rows // RPC
    rloc = rows % RPC
    chunk = rloc // CH
    rowrel = rloc % CH
    b = cols >> 15
    col_local = (cols & (BLK - 1)).astype(np.int16)

    seg = ((core * NCH + chunk) * NBLK + b)
    order = np.argsort(seg, kind="stable")
    seg_s = seg[order]
    nseg = NC * NCH * NBLK
    cnt = np.bincount(seg_s, minlength=nseg)
    gb = (cnt.reshape(NC, NCH, NBLK).max(axis=(0, 1)) + 127) // 128  # [4]
    cap = (gb * 128).astype(np.int64)                                # [4]

    seg_start = np.zeros(nseg + 1, np.int64)
    np.cumsum(cnt, out=seg_start[1:])
    pad_start = np.tile(np.concatenate([[0], np.cumsum(cap)[:-1]]), NC * NCH)
    pad_start += np.repeat(np.arange(NC * NCH) * cap.sum(), NBLK)
    pos = np.arange(len(seg_s)) - seg_start[seg_s] + pad_start[seg_s]

    tot = NC * NCH * cap.sum()
    c_arr = np.zeros(tot, np.int16)
    v_arr = np.zeros(tot, bf16)
    r_arr = np.zeros(tot, bf16)
    c_arr[pos] = col_local[order]
    v_arr[pos] = np.asarray(val, np.float32)[order].astype(bf16)
    r_arr[pos] = rowrel[order].astype(np.float32).astype(bf16)
    per_core = tot // NC
    sh = (NC, NCH, per_core // NCH)
    return (c_arr.reshape(sh), v_arr.reshape(sh), r_arr.reshape(sh),
            tuple(int(g) for g in gb))


def _preprocess(x, a1_idx, a1_val, a2_idx, a2_val, w_embed, w_classify):
    c1, v1, r1, gb1 = _pack_adjacency(a1_idx, a1_val)
    c2, v2, r2, gb2 = _pack_adjacency(a2_idx, a2_val)
    G1, G2 = sum(gb1), sum(gb2)
    G = G1 + G2

    xT = np.asarray(x, np.float32).T.astype(bf16)          # [512, 100000]
    we = np.asarray(w_embed, np.float32).astype(bf16)      # [512, 64]
    wc = np.zeros((128, 4, CLS), bf16)
    wcf = np.asarray(w_classify, np.float32)
    wc[0:64, 0] = wcf[0:64].astype(bf16)
    wc[:, 1] = wcf[64:192].astype(bf16)
    wc[:, 2] = wcf[192:320].astype(bf16)
    wc[:, 3] = wcf[320:448].astype(bf16)

    in_maps = []
    for k in range(NC):
        # idx: per chunk [128, G*8] int16 (16-partition wrap, replicated x8)
        idx_c = np.concatenate([c1[k], c2[k]], axis=1)      # [NCH, G*128]
        idx_c = idx_c.reshape(NCH, G * 8, 16).transpose(0, 2, 1)  # [NCH,16,G*8]
        idx_c = np.tile(idx_c, (1, 8, 1))                   # [NCH,128,G*8]
        # meta: [NCH, 128, 2G] = [val | rowrel], token g*128+p at [p, g]
        v = np.concatenate([v1[k], v2[k]], axis=1).reshape(NCH, G, 128)
        r = np.concatenate([r1[k], r2[k]], axis=1).reshape(NCH, G, 128)
        meta = np.concatenate([v.transpose(0, 2, 1), r.transpose(0, 2, 1)],
                              axis=2)                       # [NCH,128,2G]
        in_maps.append({
            "xT": np.ascontiguousarray(xT[:, k * RPC:(k + 1) * RPC]),
            "we": we,
            "wc": wc,
            "edge_idx": np.ascontiguousarray(idx_c),
            "edge_meta": np.ascontiguousarray(meta),
        })
    return in_maps, (gb1, gb2)


# ------------------------------------------------------------- bass program

def _build_program(gb1, gb2):
    import concourse.bass as bass
    import concourse.mybir as mybir
    from concourse import bacc, tile

    G1, G2 = sum(gb1), sum(gb2)
    G = G1 + G2
    dt = mybir.dt

    nc = bacc.Bacc(None, target_bir_lowering=False,
                   num_swdge_queues=NQUEUES)
    xT_t = nc.dram_tensor("xT", [FEAT, RPC], dt.bfloat16, kind="ExternalInput")
    we_t = nc.dram_tensor("we", [FEAT, HID], dt.bfloat16, kind="ExternalInput")
    wc_t = nc.dram_tensor("wc", [128, 4, CLS], dt.bfloat16, kind="ExternalInput")
    idx_t = nc.dram_tensor("edge_idx", [NCH, 128, G * 8], dt.int16,
                           kind="ExternalInput")
    meta_t = nc.dram_tensor("edge_meta", [NCH, 128, 2 * G], dt.bfloat16,
                            kind="ExternalInput")
    out_t = nc.dram_tensor("out", [RPC, CLS], dt.float32, kind="ExternalOutput")
    dbg = {}
    if DEBUG_DUMPS:
        dbg["dbg_h"] = nc.dram_tensor("dbg_h", [RPC, 128], dt.bfloat16,
                                      kind="ExternalOutput")
        dbg["dbg_s1"] = nc.dram_tensor("dbg_s1", [RPC, 128], dt.bfloat16,
                                       kind="ExternalOutput")
        dbg["dbg_s2"] = nc.dram_tensor("dbg_s2", [RPC, 256], dt.bfloat16,
                                       kind="ExternalOutput")

    with tile.TileContext(nc) as tc:
        with tc.tile_pool(name="dram", bufs=1, space="DRAM") as dram:
            h_loc = dram.tile([RPC, 128], dt.bfloat16)
            h_full = dram.tile([N, 128], dt.bfloat16, addr_space="Shared")
            s1_loc = dram.tile([RPC, 128], dt.bfloat16)
            s1_full = dram.tile([N, 128], dt.bfloat16, addr_space="Shared")
            s2_loc = dram.tile([RPC, 256], dt.bfloat16)

            _embed(nc, tc, xT_t, we_t, h_loc)
            if _stage_on("agh"):
                nc.gpsimd.collective_compute(
                    "AllGather", mybir.AluOpType.bypass,
                    replica_groups=[list(range(NC))],
                    ins=[h_loc[:]], outs=[h_full[:]])
            if _stage_on("hop1"):
                _hop(nc, tc, "h1", gb1, gb2, idx_t, meta_t, h_full, s1_loc,
                     src_w=64, dst_w=64)
            if _stage_on("ags1"):
                nc.gpsimd.collective_compute(
                    "AllGather", mybir.AluOpType.bypass,
                    replica_groups=[list(range(NC))],
                    ins=[s1_loc[:]], outs=[s1_full[:]])
            if _stage_on("hop2"):
                _hop(nc, tc, "h2", gb1, gb2, idx_t, meta_t, s1_full, s2_loc,
                     src_w=128, dst_w=128)
            if _stage_on("full"):
                _classify(nc, tc, wc_t, h_loc, s1_loc, s2_loc, out_t)
            if DEBUG_DUMPS:
                nc.sync.dma_start(out=dbg["dbg_h"][:], in_=h_loc[:])
                nc.sync.dma_start(out=dbg["dbg_s1"][:], in_=s1_loc[:])
                nc.sync.dma_start(out=dbg["dbg_s2"][:], in_=s2_loc[:])

    nc.compile()
    return nc


def _embed(nc, tc, xT_t, we_t, h_loc):
    import concourse.mybir as mybir
    dt = mybir.dt
    with (
        tc.tile_pool(name="e_pool", bufs=3) as pool,
        tc.tile_pool(name="e_wp", bufs=1) as wp,
        tc.tile_pool(name="e_psum", bufs=2, space="PSUM") as psum,
    ):
        we_sb = wp.tile([128, 4, HID], dt.bfloat16)
        nc.sync.dma_start(out=we_sb[:],
                          in_=we_t.rearrange("(a p) h -> p a h", p=128))
        for t in range(NCH):
            r0 = t * CH
            R = min(CH, RPC - r0)
            xt_sb = pool.tile([128, 4, CH], dt.bfloat16, tag="xt")
            nc.sync.dma_start(
                out=xt_sb[:, :, 0:R],
                in_=xT_t.rearrange("(a p) r -> p a r", p=128)[:, :, r0:r0 + R])
            h_ps = psum.tile([CH, HID], dt.float32, tag="hps")
            for a in range(4):
                nc.tensor.matmul(h_ps[0:R], xt_sb[:, a, 0:R], we_sb[:, a, :],
                                 start=(a == 0), stop=(a == 3))
            h_sb = pool.tile([CH, 128], dt.bfloat16, tag="hsb")
            nc.scalar.activation(h_sb[0:R, 0:HID], h_ps[0:R],
                                 mybir.ActivationFunctionType.Relu)
            nc.sync.dma_start(out=h_loc[r0:r0 + R, :], in_=h_sb[0:R])


def _hop(nc, tc, name, gb1, gb2, idx_t, meta_t, src_full, dst_loc, src_w, dst_w):
    """One propagation hop: for each dest chunk, gather sources, scale,
    one-hot matmul-reduce (A1 -> psum[:, :dst_w], A2 -> psum[:, dst_w:]),
    relu, store."""
    import concourse.mybir as mybir
    dt = mybir.dt
    G1, G2 = sum(gb1), sum(gb2)
    G = G1 + G2
    with (
        tc.tile_pool(name=f"{name}_pool", bufs=HOPBUFS) as pool,
        tc.tile_pool(name=f"{name}_cpool", bufs=1) as cpool,
        tc.tile_pool(name=f"{name}_psum", bufs=2, space="PSUM") as psum,
    ):
        iota_sb = cpool.tile([128, 128], dt.bfloat16)
        nc.gpsimd.iota(iota_sb[:], pattern=[[1, 128]], base=0,
                       channel_multiplier=0,
                       allow_small_or_imprecise_dtypes=True)
        for c in range(NCH):
            r0 = c * CH
            R = min(CH, RPC - r0)
            idx_sb = pool.tile([128, G * 8], dt.int16, tag="idx")
            meta_sb = pool.tile([128, 2 * G], dt.bfloat16, tag="meta")
            gath = pool.tile([128, G, 128], dt.bfloat16, tag="gath")
            P = pool.tile([128, G, 128], dt.bfloat16, tag="P")
            nc.sync.dma_start(out=idx_sb[:], in_=idx_t[c])
            nc.sync.dma_start(out=meta_sb[:], in_=meta_t[c])
            off = 0
            qn = 0
            for b, gb in enumerate(gb1 + gb2):
                while gb > 0:
                    g = gb if not SINGLE_PACKET else min(gb, 8)
                    if HOP_MODE != "no_gather":
                        nc.gpsimd.dma_gather(
                            out_ap=gath[:, off:off + g, :],
                            in_ap=src_full[b % NBLK * BLK:, :],
                            idxs_ap=idx_sb[:, off * 8:(off + g) * 8],
                            num_idxs=g * 128, num_idxs_reg=g * 128,
                            elem_size=128, queue_num=qn % NQUEUES,
                            single_packet=SINGLE_PACKET)
                        qn += 1
                    off += g
                    gb -= g
            if HOP_MODE == "no_gather":
                nc.vector.memset(gath[:, 0:1, :], 0.0)
            if HOP_MODE == "gather_only":
                continue
            nc.vector.tensor_tensor(
                out=P[:],
                in0=iota_sb[:, None, :].broadcast_to([128, G, 128]),
                in1=meta_sb[:, G:2 * G, None].broadcast_to([128, G, 128]),
                op=mybir.AluOpType.is_equal)
            nc.vector.tensor_tensor(
                out=gath[:],
                in0=gath[:],
                in1=meta_sb[:, 0:G, None].broadcast_to([128, G, 128]),
                op=mybir.AluOpType.mult)
            acc = psum.tile([CH, 2 * dst_w], dt.float32, tag="acc")
            for g in range(G):
                a1 = g < G1
                nc.tensor.matmul(
                    acc[:, 0:dst_w] if a1 else acc[:, dst_w:2 * dst_w],
                    P[:, g, :], gath[:, g, 0:src_w],
                    start=(g == 0 or g == G1),
                    stop=(g == G1 - 1 or g == G - 1))
            o_sb = pool.tile([CH, 2 * dst_w], dt.bfloat16, tag="osb")
            nc.scalar.activation(o_sb[0:R], acc[0:R],
                                 mybir.ActivationFunctionType.Relu)
            nc.sync.dma_start(out=dst_loc[r0:r0 + R, :], in_=o_sb[0:R])


def _classify(nc, tc, wc_t, h_loc, s1_loc, s2_loc, out_t):
    import concourse.mybir as mybir
    dt = mybir.dt
    with (
        tc.tile_pool(name="c_pool", bufs=3) as pool,
        tc.tile_pool(name="c_wp", bufs=1) as wp,
        tc.tile_pool(name="c_psum", bufs=2, space="PSUM") as psum,
    ):
        wc_sb = wp.tile([128, 4, CLS], dt.bfloat16)
        nc.sync.dma_start(out=wc_sb[:], in_=wc_t[:])
        ident = wp.tile([128, 128], dt.bfloat16)
        icol = wp.tile([128, 1], dt.float32)
        irow = wp.tile([128, 128], dt.bfloat16)
        nc.gpsimd.iota(irow[:], pattern=[[1, 128]], base=0,
                       channel_multiplier=0, allow_small_or_imprecise_dtypes=True)
        nc.gpsimd.iota(icol[:], pattern=[[1, 1]], base=0,
                       channel_multiplier=1, allow_small_or_imprecise_dtypes=True)
        nc.vector.tensor_scalar(ident[:], irow[:], icol[:], None,
                                mybir.AluOpType.is_equal)
        for t in range(NCH):
            r0 = t * CH
            R = min(CH, RPC - r0)
            h_sb = pool.tile([CH, HID], dt.bfloat16, tag="chs")
            s1_sb = pool.tile([CH, 128], dt.bfloat16, tag="cs1")
            s2_sb = pool.tile([CH, 256], dt.bfloat16, tag="cs2")
            if R < CH:  # last partial tile: clear stale rows
                nc.vector.memset(h_sb[:], 0.0)
                nc.vector.memset(s1_sb[:], 0.0)
                nc.vector.memset(s2_sb[:], 0.0)
            nc.sync.dma_start(out=h_sb[0:R], in_=h_loc[r0:r0 + R, 0:HID])
            nc.sync.dma_start(out=s1_sb[0:R], in_=s1_loc[r0:r0 + R, :])
            nc.sync.dma_start(out=s2_sb[0:R], in_=s2_loc[r0:r0 + R, :])
            tp_ps = psum.tile([128, 4, CH], dt.bfloat16, tag="ctp")
            nc.tensor.transpose(tp_ps[0:HID, 0, :], h_sb[:], ident[:])
            nc.tensor.transpose(tp_ps[:, 1, :], s1_sb[:], ident[:])
            nc.tensor.transpose(tp_ps[:, 2, :], s2_sb[:, 0:128], ident[:])
            nc.tensor.transpose(tp_ps[:, 3, :], s2_sb[:, 128:256], ident[:])
            tp = pool.tile([128, 4, CH], dt.bfloat16, tag="ctps")
            nc.scalar.activation(tp[0:HID, 0, :], tp_ps[0:HID, 0, :],
                                 mybir.ActivationFunctionType.Copy)
            nc.scalar.activation(tp[:, 1:4, :], tp_ps[:, 1:4, :],
                                 mybir.ActivationFunctionType.Copy)
            lg = psum.tile([CH, CLS], dt.float32, tag="clg")
            for j in range(4):
                cdim = HID if j == 0 else 128
                nc.tensor.matmul(lg[:], tp[0:cdim, j, :], wc_sb[0:cdim, j, :],
                                 start=(j == 0), stop=(j == 3))
            mx = pool.tile([CH, 1], dt.float32, tag="cmx")
            tt = pool.tile([CH, CLS], dt.float32, tag="ctt")
            ee = pool.tile([CH, CLS], dt.float32, tag="cee")
            sm = pool.tile([CH, 1], dt.float32, tag="csm")
            ll = pool.tile([CH, 1], dt.float32, tag="cll")
            oo = pool.tile([CH, CLS], dt.float32, tag="coo")
            nc.vector.tensor_reduce(mx[:], lg[:], mybir.AxisListType.X,
                                    mybir.AluOpType.max)
            nc.vector.tensor_scalar(tt[:], lg[:], mx[:], None,
                                    mybir.AluOpType.subtract)
            nc.scalar.activation(ee[:], tt[:],
                                 mybir.ActivationFunctionType.Exp,
                                 accum_out=sm[:])
            nc.scalar.activation(ll[:], sm[:],
                                 mybir.ActivationFunctionType.Ln)
            nc.vector.tensor_scalar(oo[:], tt[:], ll[:], None,
                                    mybir.AluOpType.subtract)
            nc.sync.dma_start(out=out_t[r0:r0 + R, :], in_=oo[0:R])


# ---------------------------------------------------------------- execution

def _get_runner(key):
    key = (key, STAGES, DEBUG_DUMPS, HOP_MODE, NQUEUES, HOPBUFS)
    return _get_runner_impl(key)


def _get_runner_impl(key):
    """Build (once) the compiled SPMD callable for the given group counts.

    Returns dict with the jitted sharded fn and metadata. The callable takes
    (concat_inputs..., concat_zero_outputs...) and returns concat outputs."""
    if key in _cache:
        return _cache[key]
    import jax
    import concourse.bass2jax as bass2jax
    from concourse.bass2jax import _bass_exec_p, partition_id_tensor
    from jax.sharding import Mesh, PartitionSpec
    from jax.experimental.shard_map import shard_map

    (gb1, gb2) = key[0]
    nc = _build_program(gb1, gb2)
    bass2jax.install_neuronx_cc_hook()

    partition_name = (nc.partition_id_tensor.name
                      if nc.partition_id_tensor else None)
    in_names = ["xT", "we", "wc", "edge_idx", "edge_meta"]
    out_names = ["out"]
    out_avals = [jax.core.ShapedArray((RPC, CLS), np.float32)]
    if DEBUG_DUMPS:
        out_names += ["dbg_h", "dbg_s1", "dbg_s2"]
        out_avals += [jax.core.ShapedArray((RPC, 128), ml_dtypes.bfloat16),
                      jax.core.ShapedArray((RPC, 128), ml_dtypes.bfloat16),
                      jax.core.ShapedArray((RPC, 256), ml_dtypes.bfloat16)]
    all_names = in_names + out_names + ([partition_name] if partition_name else [])
    n_params = len(in_names)

    def _body(*args):
        operands = list(args)
        if partition_name is not None:
            operands.append(partition_id_tensor())
        outs = _bass_exec_p.bind(
            *operands,
            out_avals=tuple(out_avals),
            in_names=tuple(all_names),
            out_names=tuple(out_names),
            lowering_input_output_aliases=(),
            sim_require_finite=True, sim_require_nnan=True, nc=nc)
        return tuple(outs)

    def _body_k(k):
        def f(*args):
            ins, z = args[:n_params], args[n_params:]
            outs = _body(*ins, *z)
            for _ in range(k - 1):
                z2 = tuple(o * 0 for o in outs)
                outs = _body(*ins, *z2)
            return outs
        return f

    devices = jax.devices()[:NC]
    mesh = Mesh(np.asarray(devices), ("core",))
    n_outs = len(out_names)

    def _sharded(fn):
        return jax.jit(
            shard_map(fn, mesh=mesh,
                      in_specs=(PartitionSpec("core"),) * (n_params + n_outs),
                      out_specs=(PartitionSpec("core"),) * n_outs,
                      check_rep=False),
            donate_argnums=tuple(range(n_params, n_params + n_outs)),
            keep_unused=True)

    runner = {
        "mesh": mesh,
        "n_params": n_params,
        "in_names": in_names,
        "out_avals": out_avals,
        "fn": _sharded(_body),
        "fn_k": lambda k: _sharded(_body_k(k)),
    }
    _cache[key] = runner
    return runner


def _concat_inputs(in_maps, in_names):
    return [np.concatenate([in_maps[c][n] for c in range(NC)], axis=0)
            for n in in_names]


def kernel(x, a1_idx, a1_val, a2_idx, a2_val, w_embed, w_classify):
    in_maps, key = _preprocess(x, a1_idx, a1_val, a2_idx, a2_val,
                               w_embed, w_classify)
    runner = _get_runner(key)
    concat_in = _concat_inputs(in_maps, runner["in_names"])
    zeros = [np.zeros((NC * a.shape[0], *a.shape[1:]), a.dtype)
             for a in runner["out_avals"]]
    out = runner["fn"](*concat_in, *zeros)
    if DEBUG_DUMPS:
        return tuple(np.asarray(o) for o in out)
    return np.asarray(out[0])
